# revision 15
# baseline (speedup 1.0000x reference)
"""Distributed GPT-2 attention block for 8 TRN2 NeuronCores.

Sharding: core i handles heads {2i, 2i+1} for BOTH batches (tensor-parallel
column split of c_attn). After attention, one 8-core AllToAll converts
head-sharding to token-sharding (512-token slice of the flattened [4096]
token axis per core), then each core runs c_proj (full 1024-feature
contraction) for its slice. Host unshard is pure concatenation.

Layout notes: hidden_states is passed pre-transposed [NX, B*S] (bf16, the
kernel's compute dtype) so q,k stay in [d, t] layout; scores are computed
as S^T = kT.T @ qT; the softmax denominator comes for free from an appended
ones-column in v during the PV matmul; causal masking = block skipping +
affine_select zeroing on diagonal tiles; exp and the 1/sqrt(d) scale are
fused into one ACT pass. All matmuls accumulate in f32 PSUM.
"""

import numpy as np
import ml_dtypes
from contextlib import ExitStack

import concourse.bass as bass
import concourse.bacc as bacc
import concourse.mybir as mybir
import concourse.tile as tile
from concourse.bass_utils import run_bass_kernel_spmd

B, S, NX = 2, 2048, 1024
H, D = 16, 64
HPC = 2              # heads per core
GF = HPC * D         # 128 features per head group
NCORES = 8
SF = B * S           # 4096 flattened tokens
TSL = SF // NCORES   # 512-token output slice per core

F32 = mybir.dt.float32
BF16 = mybir.dt.bfloat16


def build(zero_attn_bias: bool, zero_proj_bias: bool) -> bass.Bass:
    nc = bacc.Bacc(None)

    hst = nc.declare_dram_parameter("hst", [NX, SF], BF16, isOutput=False)
    wqkv = nc.declare_dram_parameter("wqkv", [NX, 3 * GF], BF16, isOutput=False)
    bqkv = nc.declare_dram_parameter("bqkv", [3 * GF, 1], F32, isOutput=False)
    wproj = nc.declare_dram_parameter("wproj", [NX, NX], BF16, isOutput=False)
    bproj = nc.declare_dram_parameter("bproj", [NX, 1], F32, isOutput=False)
    out_ext = nc.declare_dram_parameter("out", [NX, TSL], F32, isOutput=True)

    KT = NX // 128   # 8 k tiles

    with tile.TileContext(nc) as tc, ExitStack() as ctx:
        pool1 = ctx.enter_context(tc.tile_pool(name="persist", bufs=1))
        small = ctx.enter_context(tc.tile_pool(name="small", bufs=2))
        ppool = ctx.enter_context(tc.tile_pool(name="ppool", bufs=2))
        psum = ctx.enter_context(tc.tile_pool(name="psum", bufs=2, space="PSUM"))
        psum_av = ctx.enter_context(tc.tile_pool(name="psum_av", bufs=2, space="PSUM"))
        psum_rb = ctx.enter_context(tc.tile_pool(name="psum_rb", bufs=2, space="PSUM"))
        dram = ctx.enter_context(tc.tile_pool(name="dram", bufs=1, space="DRAM"))

        # ---- load weights and hidden states (bf16, direct, no staging) ------
        # 3D-AP DMAs: [part, ktile, col] <- DRAM[ktile*128 + part, col]
        wqkv_bf = pool1.tile([128, KT, 3 * GF], BF16)
        nc.sync.dma_start(
            wqkv_bf[:], wqkv[:, :].rearrange("(kt p) c -> p kt c", p=128))
        wproj_bf = pool1.tile([128, KT, NX], BF16)
        for half in range(2):
            sl = slice(half * (KT // 2), (half + 1) * (KT // 2))
            nc.sync.dma_start(
                wproj_bf[:, sl, :],
                wproj[:, :].rearrange("(kt p) c -> p kt c", p=128)[:, sl, :])
        hst_bf = pool1.tile([128, KT, SF], BF16)
        for kt in range(KT):
            nc.sync.dma_start(
                hst_bf[:, kt, :], hst[kt * 128:(kt + 1) * 128, :])

        # biases as per-partition tiles (q/k: feature-per-partition in qkT layout)
        bqk_t = pool1.tile([128, 2, 1], F32)   # ft 0 = q(2 heads), ft 1 = k
        if not zero_attn_bias:
            for ft in range(2):
                nc.sync.dma_start(bqk_t[:, ft, :], bqkv[ft * 128:(ft + 1) * 128, :])
        bv_t = pool1.tile([64, HPC, 1], F32)
        if not zero_attn_bias:
            for h in range(HPC):
                nc.sync.dma_start(
                    bv_t[:, h, :], bqkv[2 * GF + h * D:2 * GF + (h + 1) * D, :])
        bproj_t = pool1.tile([128, KT, 1], F32)
        if not zero_proj_bias:
            nc.sync.dma_start(
                bproj_t[:], bproj[:, :].rearrange("(kt p) c -> p kt c", p=128))

        # ---- QKV projection --------------------------------------------------
        # q,k transposed: qk_sb[:, ft, t]; ft 0 = q (2 heads), ft 1 = k
        qk_sb = pool1.tile([128, 2, SF], BF16)
        for ft in range(2):
            for tch in range(SF // 512):
                ps = psum.tile([128, 2, 512], F32, tag="mm")
                for kt in range(KT):
                    nc.tensor.matmul(
                        ps[:, 0, :],
                        lhsT=wqkv_bf[:, kt, ft * 128:(ft + 1) * 128],
                        rhs=hst_bf[:, kt, tch * 512:(tch + 1) * 512],
                        start=(kt == 0), stop=(kt == KT - 1),
                    )
                if zero_attn_bias:
                    nc.scalar.copy(qk_sb[:, ft, tch * 512:(tch + 1) * 512], ps[:, 0, :])
                else:
                    nc.scalar.activation(
                        qk_sb[:, ft, tch * 512:(tch + 1) * 512], ps[:, 0, :],
                        mybir.ActivationFunctionType.Identity, bias=bqk_t[:, ft, :])

        # v natural [token, feat] + ones column: v_sb [128, tt, h, 65]
        v_sb = pool1.tile([128, SF // 128, HPC, D + 1], BF16)
        for tt in range(SF // 128):
            ps = psum.tile([128, 2, 512], F32, tag="mm")
            for kt in range(KT):
                nc.tensor.matmul(
                    ps[:, 0, 0:GF],
                    lhsT=hst_bf[:, kt, tt * 128:(tt + 1) * 128],
                    rhs=wqkv_bf[:, kt, 2 * GF:3 * GF],
                    start=(kt == 0), stop=(kt == KT - 1),
                )
            nc.vector.tensor_copy(
                v_sb[:, tt, :, 0:D],
                ps[:, 0, 0:GF].rearrange("p (h d) -> p h d", h=HPC))
        nc.gpsimd.memset(v_sb[:, :, :, D:D + 1], 1.0)

        # ---- attention -------------------------------------------------------
        a2a_in = dram.tile([NCORES, GF, TSL], BF16)
        a2a_out = dram.tile([NCORES, GF, TSL], BF16)

        ones1 = pool1.tile([1, D], F32)
        nc.gpsimd.memset(ones1[:], 1.0)

        # staging for all heads'/blocks' normalized attention outputs:
        # [64 part, h, slot(=4b+tb), 512] -> one DMA per h to a2a_in
        atall = pool1.tile([D, HPC, NCORES, TSL], BF16)

        for h in range(HPC):
            qrow = 64 * h
            for b in range(B):
                tok0 = b * S               # batch token offset (flattened)
                tt0 = tok0 // 128          # v tile offset
                for tb in range(4):        # 512-query blocks within the batch
                    ntj = 4 * (tb + 1)     # causal: tj tiles 0..ntj-1
                    av = psum_av.tile([D + 1, 512], F32, tag="av")
                    for tjq in range(ntj // 4):   # quads of tj tiles
                        st_ps = psum.tile([128, 2, 512], F32, tag="mm")
                        pt = ppool.tile([128, 4, 512], BF16, tag="pt")
                        for u in range(4):
                            t = 4 * tjq + u
                            nc.tensor.matmul(
                                st_ps[:, u % 2, :],
                                lhsT=qk_sb[qrow:qrow + 64, 1,
                                           tok0 + t * 128:tok0 + (t + 1) * 128],
                                rhs=qk_sb[qrow:qrow + 64, 0,
                                          tok0 + tb * 512:tok0 + (tb + 1) * 512],
                                start=True, stop=True,
                            )
                            # exp((q.k)/sqrt(d)); pairs share one psum tile
                            if u % 2 == 1:
                                nc.scalar.activation(
                                    pt[:, u - 1:u + 1, :], st_ps[:],
                                    mybir.ActivationFunctionType.Exp, scale=0.125)
                                if u == 1:
                                    st_ps = psum.tile([128, 2, 512], F32, tag="mm")
                        if tjq == ntj // 4 - 1:
                            # diagonal quad: zero entries where key > query
                            for u in range(4):
                                t = 4 * tjq + u
                                nc.gpsimd.affine_select(
                                    out=pt[:, u, :], in_=pt[:, u, :],
                                    compare_op=mybir.AluOpType.is_ge, fill=0.0,
                                    base=tb * 512 - t * 128,
                                    pattern=[[1, 512]], channel_multiplier=-1)
                        for u in range(4):
                            t = 4 * tjq + u
                            nc.tensor.matmul(
                                av[:],
                                lhsT=v_sb[:, tt0 + t, h, :],
                                rhs=pt[:, u, :],
                                start=(t == 0), stop=(t == ntj - 1),
                            )
                    # normalize by row D (the P row-sums); broadcast via PE
                    recip = small.tile([1, 512], F32, tag="recip")
                    nc.vector.reciprocal(recip[:], av[D:D + 1, :])
                    rb = psum_rb.tile([D, 512], F32, tag="rb")
                    nc.tensor.matmul(rb[:], lhsT=ones1[:], rhs=recip[:],
                                     start=True, stop=True)
                    rb_sb = ppool.tile([D, 512], F32, tag="rbs")
                    nc.scalar.copy(rb_sb[:], rb[:])
                    dst = atall[:, h, 4 * b + tb, :]
                    if zero_attn_bias:
                        nc.vector.tensor_mul(dst, av[0:D, :], rb_sb[:])
                    else:
                        at = ppool.tile([D, 512], BF16, tag="at")
                        nc.vector.tensor_mul(at[:], av[0:D, :], rb_sb[:])
                        nc.scalar.activation(
                            dst, at[:],
                            mybir.ActivationFunctionType.Identity, bias=bv_t[:, h, :])
            # one store per head: [64, slot, 512] -> a2a_in[slot, h*64:(h+1)*64, :]
            nc.sync.dma_start(
                a2a_in[:, qrow:qrow + 64, :].rearrange("s p c -> p s c"),
                atall[:, h, :, :])

        nc.gpsimd.collective_compute(
            "AllToAll",
            mybir.AluOpType.bypass,
            ins=[a2a_in.opt()],
            outs=[a2a_out.opt()],
            replica_groups=[list(range(NCORES))],
        )

        # ---- c_proj over the received [NX, TSL] block -----------------------
        art_bf = pool1.tile([128, KT, TSL], BF16)
        for half in range(2):
            sl = slice(half * (KT // 2), (half + 1) * (KT // 2))
            nc.sync.dma_start(
                art_bf[:, sl, :],
                a2a_out[:].rearrange("ft p c -> p ft c")[:, sl, :])
        otall = pool1.tile([128, KT, TSL], F32)
        for ntile in range(KT):
            ps = psum.tile([128, 2, 512], F32, tag="mm")
            for ft in range(KT):
                nc.tensor.matmul(
                    ps[:, 0, :],
                    lhsT=wproj_bf[:, ft, ntile * 128:(ntile + 1) * 128],
                    rhs=art_bf[:, ft, :],
                    start=(ft == 0), stop=(ft == KT - 1),
                )
            if zero_proj_bias:
                nc.scalar.copy(otall[:, ntile, :], ps[:, 0, :])
            else:
                nc.scalar.activation(
                    otall[:, ntile, :], ps[:, 0, :],
                    mybir.ActivationFunctionType.Identity, bias=bproj_t[:, ntile, :])
        for half in range(2):
            sl = slice(half * (KT // 2), (half + 1) * (KT // 2))
            nc.sync.dma_start(
                out_ext[:, :].rearrange("(nt p) c -> p nt c", p=128)[:, sl, :],
                otall[:, sl, :])

    nc.finalize()
    return nc


_CACHE = {}


def _get_nc(zero_attn_bias, zero_proj_bias):
    key = (zero_attn_bias, zero_proj_bias)
    if key not in _CACHE:
        _CACHE[key] = build(*key)
    return _CACHE[key]


def kernel(hidden_states, c_attn_w, c_attn_b, c_proj_w, c_proj_b, **extra):
    hidden_states = np.asarray(hidden_states, np.float32)
    c_attn_w = np.asarray(c_attn_w, np.float32)
    c_attn_b = np.asarray(c_attn_b, np.float32)
    c_proj_w = np.asarray(c_proj_w, np.float32)
    c_proj_b = np.asarray(c_proj_b, np.float32)

    zero_attn_bias = not np.any(c_attn_b)
    zero_proj_bias = not np.any(c_proj_b)
    nc = _get_nc(zero_attn_bias, zero_proj_bias)

    bf = ml_dtypes.bfloat16
    # [NX, B*S] pre-transposed hidden states in the kernel's compute dtype
    hsT = np.ascontiguousarray(hidden_states.reshape(B * S, NX).T).astype(bf)
    wproj_bf = np.ascontiguousarray(c_proj_w).astype(bf)
    bproj = np.ascontiguousarray(c_proj_b.reshape(NX, 1))

    in_maps = []
    for i in range(NCORES):
        cols = np.r_[i * GF:(i + 1) * GF,
                     NX + i * GF:NX + (i + 1) * GF,
                     2 * NX + i * GF:2 * NX + (i + 1) * GF]
        in_maps.append({
            "hst": hsT,
            "wqkv": np.ascontiguousarray(c_attn_w[:, cols]).astype(bf),
            "bqkv": np.ascontiguousarray(c_attn_b[cols].reshape(3 * GF, 1)),
            "wproj": wproj_bf,
            "bproj": bproj,
        })

    res = run_bass_kernel_spmd(nc, in_maps, core_ids=list(range(NCORES)))
    out = np.empty((B * S, NX), np.float32)
    for i in range(NCORES):
        out[i * TSL:(i + 1) * TSL, :] = res.results[i]["out"].T
    return out.reshape(B, S, NX)


if __name__ == "__main__":
    rng = np.random.default_rng(0)
    hs = rng.standard_normal((B, S, NX), dtype=np.float32)
    wa = (rng.standard_normal((NX, 3 * NX), dtype=np.float32) * 0.02)
    wp = (rng.standard_normal((NX, NX), dtype=np.float32) * 0.02)
    o = kernel(hidden_states=hs, c_attn_w=wa, c_attn_b=np.zeros(3 * NX, np.float32),
               c_proj_w=wp, c_proj_b=np.zeros(NX, np.float32))
    print(o.shape, o.dtype)
